# Initial kernel scaffold
#
"""Trainium2 Bass kernel for nn_Autoencoder_65223373357102 (FLAME-style autoencoder).

Strategy:
  Phase 1 (8-way tensor parallel): encoder GEMM [64,150528]@[150528,556] sharded
  along the input-feature axis. Each core transposes its x shard on TensorE,
  multiplies against its 1/8 slice of enc_W, adds enc_b/8 via a K=1 matmul, and
  AllReduces the [64,556] latent (142 KB).
  Phase 2 (replicated): blendshape GEMM [64,400]@[400,3*5023] in plane-separated
  layout + all per-batch geometry with batch on partitions; per-batch scalars are
  broadcast along the free axis via tensor_scalar. Every core computes the full
  output; the host takes core 0's copy.
"""
import sys

sys.path.insert(0, "/opt/trn_rl_repo")

import numpy as np

from concourse import bass, mybir, tile
from concourse.bass_utils import run_bass_kernel_spmd
from concourse.masks import make_identity

F32 = mybir.dt.float32
ALU = mybir.AluOpType
ACTF = mybir.ActivationFunctionType
AX = mybir.AxisListType

B = 64
V = 5023
VM = 3500
LAT = 556
DIN = 3 * 224 * 224  # 150528
NCORES = 8
KSH = DIN // NCORES  # 18816
KTILES = KSH // 128  # 147
NOUT = 2 * VM + 68 + 11  # 7079
GAZE_DIR = -1.0
HALF_PI = 1.5707963267948966


def _chunks(total, step):
    out = []
    o = 0
    while o < total:
        out.append((o, min(step, total - o)))
        o += step
    return out


class Geo:
    """Helper for tiny per-batch scalar ops on [B,1] tiles."""

    def __init__(self, nc, pool):
        self.nc = nc
        self.pool = pool

    def t(self, cols=1):
        return self.pool.tile([B, cols], F32, tag=f"geo{cols}")

    def mul(self, a, b):
        o = self.t()
        self.nc.vector.tensor_tensor(out=o, in0=a, in1=b, op=ALU.mult)
        return o

    def add(self, a, b):
        o = self.t()
        self.nc.vector.tensor_tensor(out=o, in0=a, in1=b, op=ALU.add)
        return o

    def sub(self, a, b):
        o = self.t()
        self.nc.vector.tensor_tensor(out=o, in0=a, in1=b, op=ALU.subtract)
        return o

    def mac(self, a, s, acc):
        """(a * s) + acc, s is a [B,1] AP scalar."""
        o = self.t()
        self.nc.vector.scalar_tensor_tensor(
            out=o, in0=a, scalar=s, in1=acc, op0=ALU.mult, op1=ALU.add
        )
        return o

    def dot3(self, ax, ay, az, bx, by, bz):
        o = self.mul(ax, bx)
        o = self.mac(ay, by, o)
        o = self.mac(az, bz, o)
        return o

    def cross3(self, ax, ay, az, bx, by, bz):
        """a x b -> 3 [B,1] tiles."""
        cx = self.sub(self.mul(ay, bz), self.mul(az, by))
        cy = self.sub(self.mul(az, bx), self.mul(ax, bz))
        cz = self.sub(self.mul(ax, by), self.mul(ay, bx))
        return cx, cy, cz


def axis_angle_R(nc, g, aa3):
    """aa3: [B,3] axis-angle tile -> R [B,9] tile, R[l,i] at col l*3+i.

    R = c*I + s*K + (1-c) a a^T  (Rodrigues, matching reference)
    """
    pool = g.pool
    sq = pool.tile([B, 3], F32, tag="aaR_sq")
    nc.vector.tensor_tensor(out=sq, in0=aa3, in1=aa3, op=ALU.mult)
    th2 = g.t()
    nc.vector.tensor_reduce(out=th2, in_=sq, axis=AX.X, op=ALU.add)
    theta = g.t()
    nc.scalar.activation(out=theta, in_=th2, func=ACTF.Sqrt)
    thm = g.t()
    nc.vector.tensor_scalar_max(out=thm, in0=theta, scalar1=1e-8)
    rth = g.t()
    nc.vector.reciprocal(out=rth, in_=thm)
    axis3 = pool.tile([B, 3], F32, tag="aaR_axis")
    nc.vector.tensor_scalar_mul(out=axis3, in0=aa3, scalar1=rth)
    s = g.t()
    nc.scalar.activation(out=s, in_=theta, func=ACTF.Sin)
    c = g.t()
    nc.scalar.activation(out=c, in_=theta, func=ACTF.Sin, bias=HALF_PI)
    omc = g.t()
    nc.vector.tensor_scalar(
        out=omc, in0=c, scalar1=-1.0, scalar2=1.0, op0=ALU.mult, op1=ALU.add
    )
    ax, ay, az = axis3[:, 0:1], axis3[:, 1:2], axis3[:, 2:3]
    # diag: omc*a_i^2 + c
    asq = pool.tile([B, 3], F32, tag="aaR_asq")
    nc.vector.tensor_tensor(out=asq, in0=axis3, in1=axis3, op=ALU.mult)
    R = pool.tile([B, 9], F32, tag="aaR_R")
    diag = R.rearrange("b (r c) -> b r c", r=3)  # [B,3,3]
    # write diag entries R[0],R[4],R[8] via strided AP
    dmul = pool.tile([B, 3], F32, tag="aaR_dmul")
    nc.vector.tensor_scalar_mul(out=dmul, in0=asq, scalar1=omc)
    # s*a
    sa = pool.tile([B, 3], F32, tag="aaR_sa")
    nc.vector.tensor_scalar_mul(out=sa, in0=axis3, scalar1=s)
    sax, say, saz = sa[:, 0:1], sa[:, 1:2], sa[:, 2:3]
    # off-diag products omc*ai*aj
    mxy = g.mul(g.mul(ax, ay), omc)
    mxz = g.mul(g.mul(ax, az), omc)
    myz = g.mul(g.mul(ay, az), omc)
    # assemble
    ddiag = R[:, 0:9:4]  # cols 0,4,8 -> view shape [B,3] stride 4
    nc.vector.tensor_scalar(
        out=ddiag, in0=dmul, scalar1=c, scalar2=None, op0=ALU.add
    )
    nc.vector.tensor_tensor(out=R[:, 1:2], in0=mxy, in1=saz, op=ALU.subtract)  # R01
    nc.vector.tensor_tensor(out=R[:, 2:3], in0=mxz, in1=say, op=ALU.add)  # R02
    nc.vector.tensor_tensor(out=R[:, 3:4], in0=mxy, in1=saz, op=ALU.add)  # R10
    nc.vector.tensor_tensor(out=R[:, 5:6], in0=myz, in1=sax, op=ALU.subtract)  # R12
    nc.vector.tensor_tensor(out=R[:, 6:7], in0=mxz, in1=say, op=ALU.subtract)  # R20
    nc.vector.tensor_tensor(out=R[:, 7:8], in0=myz, in1=sax, op=ALU.add)  # R21
    _ = diag
    return R


def build_graph(fl_idx, idx4, idx2, l_lo, r_lo):
    """fl_idx: 68 ints (vert cols for masked landmarks), idx4/idx2: landmark vert
    cols, l_lo/r_lo: start of the contiguous eye ranges."""
    nc = bass.Bass(target_bir_lowering=False)

    x_p = nc.declare_dram_parameter("x_sh", [B, KSH], F32, isOutput=False)
    w_p = nc.declare_dram_parameter("w_sh", [KSH, LAT], F32, isOutput=False)
    b_p = nc.declare_dram_parameter("enc_b", [1, LAT], F32, isOutput=False)
    tpl_p = nc.declare_dram_parameter("tmpl", [3, V], F32, isOutput=False)
    bas_p = nc.declare_dram_parameter("basis", [400, 3, V], F32, isOutput=False)
    cam_p = nc.declare_dram_parameter("cam", [B, 12], F32, isOutput=False)
    out_p = nc.declare_dram_parameter("out", [B, 3, NOUT], F32, isOutput=True)

    ar_in = nc.dram_tensor("ar_in", [B, LAT], F32)
    ar_out = nc.dram_tensor("ar_out", [B, LAT], F32, addr_space="Shared")

    with tile.TileContext(nc) as tc:
        with (
            tc.tile_pool(name="consts", bufs=1) as consts,
            tc.tile_pool(name="latents", bufs=1) as latp,
            tc.tile_pool(name="geo", bufs=1) as geop,
            tc.tile_pool(name="planes", bufs=1) as planep,
        ):
            ident = consts.tile([128, 128], F32)
            make_identity(nc, ident)
            ones8 = consts.tile([1, B], F32)
            nc.vector.memset(ones8, 1.0 / NCORES)
            ones1 = consts.tile([1, B], F32)
            nc.vector.memset(ones1, 1.0)
            b_sb = consts.tile([1, LAT], F32)
            nc.sync.dma_start(out=b_sb, in_=b_p[:, :])

            # ---------------- Phase 1: encoder GEMM ----------------
            NSPL = [(0, 512), (512, 44)]
            with (
                tc.tile_pool(name="xin", bufs=3) as xin,
                tc.tile_pool(name="wts", bufs=6) as wts,
                tc.tile_pool(name="xtp", bufs=3, space="PSUM") as xtp,
                tc.tile_pool(name="xts", bufs=4) as xts,
                tc.tile_pool(name="encp", bufs=1, space="PSUM") as encp,
            ):
                pe = [encp.tile([B, n], F32, tag=f"pe{j}") for j, (_, n) in enumerate(NSPL)]
                XC = 7 * 128  # x chunk cols
                for ci in range(KTILES // 7):
                    x_c = xin.tile([B, XC], F32)
                    nc.sync.dma_start(out=x_c, in_=x_p[:, ci * XC:(ci + 1) * XC])
                    for t in range(7):
                        k = ci * 7 + t
                        w_t = wts.tile([128, LAT], F32)
                        nc.sync.dma_start(
                            out=w_t, in_=w_p[k * 128:(k + 1) * 128, :]
                        )
                        tp = xtp.tile([128, B], F32)
                        nc.tensor.transpose(
                            tp, x_c[:, t * 128:(t + 1) * 128], ident[:B, :B]
                        )
                        xT = xts.tile([128, B], F32)
                        nc.scalar.tensor_copy(out=xT, in_=tp)
                        for j, (n0, n) in enumerate(NSPL):
                            nc.tensor.matmul(
                                pe[j],
                                lhsT=xT,
                                rhs=w_t[:, n0:n0 + n],
                                start=(k == 0),
                                stop=False,
                            )
                for j, (n0, n) in enumerate(NSPL):
                    nc.tensor.matmul(
                        pe[j],
                        lhsT=ones8,
                        rhs=b_sb[:, n0:n0 + n],
                        start=False,
                        stop=True,
                    )
                lat1 = latp.tile([B, LAT], F32)
                for j, (n0, n) in enumerate(NSPL):
                    nc.vector.tensor_copy(out=lat1[:, n0:n0 + n], in_=pe[j])
                nc.sync.dma_start(out=ar_in[:, :], in_=lat1)

            nc.gpsimd.collective_compute(
                "AllReduce",
                ALU.add,
                replica_groups=[list(range(NCORES))],
                ins=[ar_in.ap().opt()],
                outs=[ar_out.ap().opt()],
            )
            lat = latp.tile([B, LAT], F32)
            nc.sync.dma_start(out=lat, in_=ar_out[:, :])

            # ---------------- Phase 1.5: transpose shape params ----------------
            KSPL = [(0, 128), (128, 128), (256, 128), (384, 16)]
            spT = []
            with tc.tile_pool(name="sptp", bufs=2, space="PSUM") as sptp:
                for (c0, cw) in KSPL:
                    tp = sptp.tile([128, B], F32)
                    nc.tensor.transpose(tp[:cw, :], lat[:, c0:c0 + cw], ident[:B, :B])
                    st = latp.tile([cw, B], F32, tag=f"spT{c0}")
                    nc.scalar.tensor_copy(out=st, in_=tp[:cw, :])
                    spT.append(st)

            # ---------------- Phase 2: blendshape + template ----------------
            vert = planep.tile([B, 3, V], F32)  # raw verts, plane-major
            vps = geop.tile([B, 33], F32)  # per-chunk partial sums
            VCH = _chunks(V, 512)  # 10 chunks
            with (
                tc.tile_pool(name="bas", bufs=4) as basp,
                tc.tile_pool(name="tpl", bufs=2) as tplp,
                tc.tile_pool(name="bpsum", bufs=2, space="PSUM") as bpsum,
            ):
                for p in range(3):
                    for j, (n0, n) in enumerate(VCH):
                        pv = bpsum.tile([B, 512], F32)
                        for ki, (k0, kw) in enumerate(KSPL):
                            bt = basp.tile([128, 512], F32)
                            nc.sync.dma_start(
                                out=bt[:kw, :n], in_=bas_p[k0:k0 + kw, p, n0:n0 + n]
                            )
                            nc.tensor.matmul(
                                pv[:, :n],
                                lhsT=spT[ki],
                                rhs=bt[:kw, :n],
                                start=(ki == 0),
                                stop=False,
                            )
                        tl = tplp.tile([1, 512], F32)
                        nc.sync.dma_start(out=tl[:, :n], in_=tpl_p[p:p + 1, n0:n0 + n])
                        nc.tensor.matmul(
                            pv[:, :n], lhsT=ones1, rhs=tl[:, :n], start=False, stop=True
                        )
                        nc.scalar.tensor_copy(out=vert[:, p, n0:n0 + n], in_=pv[:, :n])
                        nc.vector.tensor_reduce(
                            out=vps[:, p * 11 + j:p * 11 + j + 1],
                            in_=pv[:, :n],
                            axis=AX.X,
                            op=ALU.add,
                        )

            g = Geo(nc, geop)
            # vmean [B,3]
            vm = geop.tile([B, 3], F32)
            for p in range(3):
                nc.vector.tensor_reduce(
                    out=vm[:, p:p + 1], in_=vps[:, p * 11:p * 11 + 11],
                    axis=AX.X, op=ALU.add,
                )
            vms = geop.tile([B, 3], F32)
            nc.vector.tensor_scalar_mul(out=vms, in0=vm, scalar1=1.0 / V)

            # face rotation matrix, scaled
            aa_face = lat[:, 545:548]
            Rf = axis_angle_R(nc, g, aa_face)
            fs = g.t()  # face_scale = latent[551]+1
            nc.vector.tensor_scalar_add(out=fs, in0=lat[:, 551:552], scalar1=1.0)
            Rs = geop.tile([B, 9], F32)
            nc.vector.tensor_scalar_mul(out=Rs, in0=Rf, scalar1=fs)
            # offsets: off_i = face_t_i - sum_l vms_l*Rs[l,i]
            off = geop.tile([B, 3], F32)
            for i in range(3):
                t = g.mul(vms[:, 0:1], Rs[:, i:i + 1])
                t = g.mac(vms[:, 1:2], Rs[:, 3 + i:4 + i], t)
                t = g.mac(vms[:, 2:3], Rs[:, 6 + i:7 + i], t)
                nc.vector.tensor_tensor(
                    out=off[:, i:i + 1], in0=lat[:, 548 + i:549 + i], in1=t,
                    op=ALU.subtract,
                )

            # rotate+scale+translate all verts: rt[:,i,:] = sum_l vert[:,l,:]*Rs[l,i] + off_i
            rt = planep.tile([B, 3, V], F32)
            for i in range(3):
                nc.vector.tensor_scalar(
                    out=rt[:, i, :], in0=vert[:, 0, :],
                    scalar1=Rs[:, i:i + 1], scalar2=off[:, i:i + 1],
                    op0=ALU.mult, op1=ALU.add,
                )
                for l in (1, 2):
                    nc.vector.scalar_tensor_tensor(
                        out=rt[:, i, :], in0=vert[:, l, :],
                        scalar=Rs[:, 3 * l + i:3 * l + i + 1], in1=rt[:, i, :],
                        op0=ALU.mult, op1=ALU.add,
                    )

            # eye processing
            EW = 546
            gaze = {}
            centers = {}
            for side, lo, pivot, rcol in (
                ("l", l_lo, 4051, 552), ("r", r_lo, 4597, 554),
            ):
                # center = mean of rotated eye verts
                cc = geop.tile([B, 3], F32, tag=f"cc_{side}")
                for i in range(3):
                    nc.vector.tensor_reduce(
                        out=cc[:, i:i + 1], in_=rt[:, i, lo:lo + EW],
                        axis=AX.X, op=ALU.add,
                    )
                c3 = geop.tile([B, 3], F32, tag=f"c3_{side}")
                nc.vector.tensor_scalar_mul(out=c3, in0=cc, scalar1=1.0 / EW)
                centers[side] = c3
                # init gaze dir = normalize(vert[pivot] - c)
                a3 = geop.tile([B, 3], F32, tag=f"a3_{side}")
                for i in range(3):
                    nc.vector.tensor_tensor(
                        out=a3[:, i:i + 1], in0=rt[:, i, pivot:pivot + 1],
                        in1=c3[:, i:i + 1], op=ALU.subtract,
                    )
                sq = geop.tile([B, 3], F32, tag=f"nsq_{side}")
                nc.vector.tensor_tensor(out=sq, in0=a3, in1=a3, op=ALU.mult)
                n2 = g.t()
                nc.vector.tensor_reduce(out=n2, in_=sq, axis=AX.X, op=ALU.add)
                nn = g.t()
                nc.scalar.activation(out=nn, in_=n2, func=ACTF.Sqrt)
                rn = g.t()
                nc.vector.reciprocal(out=rn, in_=nn)
                nc.vector.tensor_scalar_mul(out=a3, in0=a3, scalar1=rn)
                ax, ay, az = a3[:, 0:1], a3[:, 1:2], a3[:, 2:3]
                # find_gaze_R: b=(0,0,GAZE_DIR)=(0,0,-1)
                # v = a x b = (-ay, ax, 0) [for GAZE_DIR=-1: v=(ay*g, -ax*g, 0)=(-ay, ax, 0)]
                vx = g.t()
                nc.vector.tensor_scalar_mul(out=vx, in0=ay, scalar1=-GAZE_DIR * -1.0)
                # careful: v = a x b with b=(0,0,g): vx = ay*g - az*0 = ay*g; vy = -ax*g
                nc.vector.tensor_scalar_mul(out=vx, in0=ay, scalar1=GAZE_DIR)
                vy = g.t()
                nc.vector.tensor_scalar_mul(out=vy, in0=ax, scalar1=-GAZE_DIR)
                # c = a.b = az*g
                cdot = g.t()
                nc.vector.tensor_scalar_mul(out=cdot, in0=az, scalar1=GAZE_DIR)
                # f = 1/(1+c+1e-8)
                fden = g.t()
                nc.vector.tensor_scalar_add(out=fden, in0=cdot, scalar1=1.0 + 1e-8)
                f = g.t()
                nc.vector.reciprocal(out=f, in_=fden)
                # vv = vx^2+vy^2
                vv = g.mac(vy, vy, g.mul(vx, vx))
                fvv = g.mul(f, vv)
                dd = g.t()  # 1 - f*vv
                nc.vector.tensor_scalar(
                    out=dd, in0=fvv, scalar1=-1.0, scalar2=1.0, op0=ALU.mult, op1=ALU.add
                )
                fxy = g.mul(g.mul(vx, vy), f)
                Rl = geop.tile([B, 9], F32, tag=f"Rl_{side}")
                # Rl00 = dd + f*vx^2 ; Rl11 = dd + f*vy^2; Rl22 = dd
                nc.vector.tensor_tensor(
                    out=Rl[:, 0:1], in0=dd, in1=g.mul(f, g.mul(vx, vx)), op=ALU.add
                )
                nc.vector.tensor_tensor(
                    out=Rl[:, 4:5], in0=dd, in1=g.mul(f, g.mul(vy, vy)), op=ALU.add
                )
                nc.vector.tensor_copy(out=Rl[:, 8:9], in_=dd)
                nc.vector.tensor_copy(out=Rl[:, 1:2], in_=fxy)  # R01 = f vx vy
                nc.vector.tensor_copy(out=Rl[:, 3:4], in_=fxy)  # R10
                nc.vector.tensor_copy(out=Rl[:, 2:3], in_=vy)  # R02 = vy
                nc.vector.tensor_scalar_mul(out=Rl[:, 5:6], in0=vx, scalar1=-1.0)  # R12
                nc.vector.tensor_scalar_mul(out=Rl[:, 6:7], in0=vy, scalar1=-1.0)  # R20
                nc.vector.tensor_copy(out=Rl[:, 7:8], in_=vx)  # R21
                # eyeball rotation from latent rot2 (2 comps, az=0)
                aa2 = geop.tile([B, 3], F32, tag=f"aa2_{side}")
                nc.vector.memset(aa2, 0.0)
                nc.vector.tensor_copy(out=aa2[:, 0:2], in_=lat[:, rcol:rcol + 2])
                R2 = axis_angle_R(nc, g, aa2)
                # gaze = GAZE_DIR * R2[2,:]
                gz = geop.tile([B, 3], F32, tag=f"gz_{side}")
                nc.vector.tensor_scalar_mul(out=gz, in0=R2[:, 6:9], scalar1=GAZE_DIR)
                gaze[side] = gz
                # M = Rl @ R2
                M = geop.tile([B, 9], F32, tag=f"M_{side}")
                for l in range(3):
                    for i in range(3):
                        t = g.mul(Rl[:, 3 * l:3 * l + 1], R2[:, i:i + 1])
                        t = g.mac(R2[:, 3 + i:4 + i], Rl[:, 3 * l + 1:3 * l + 2], t)
                        t = g.mac(R2[:, 6 + i:7 + i], Rl[:, 3 * l + 2:3 * l + 3], t)
                        nc.vector.tensor_copy(out=M[:, 3 * l + i:3 * l + i + 1], in_=t)
                # offe_i = c_i - sum_l c_l M[l,i]
                offe = geop.tile([B, 3], F32, tag=f"offe_{side}")
                for i in range(3):
                    t = g.mul(c3[:, 0:1], M[:, i:i + 1])
                    t = g.mac(c3[:, 1:2], M[:, 3 + i:4 + i], t)
                    t = g.mac(c3[:, 2:3], M[:, 6 + i:7 + i], t)
                    nc.vector.tensor_tensor(
                        out=offe[:, i:i + 1], in0=c3[:, i:i + 1], in1=t, op=ALU.subtract
                    )
                # apply to eye slice
                etmp = planep.tile([B, 3, EW], F32, tag="etmp")
                for i in range(3):
                    nc.vector.tensor_scalar(
                        out=etmp[:, i, :], in0=rt[:, 0, lo:lo + EW],
                        scalar1=M[:, i:i + 1], scalar2=offe[:, i:i + 1],
                        op0=ALU.mult, op1=ALU.add,
                    )
                    for l in (1, 2):
                        nc.vector.scalar_tensor_tensor(
                            out=etmp[:, i, :], in0=rt[:, l, lo:lo + EW],
                            scalar=M[:, 3 * l + i:3 * l + i + 1], in1=etmp[:, i, :],
                            op0=ALU.mult, op1=ALU.add,
                        )
                for i in range(3):
                    nc.vector.tensor_copy(out=rt[:, i, lo:lo + EW], in_=etmp[:, i, :])

            # face centre from landmarks
            fc = geop.tile([B, 3], F32)
            for i in range(3):
                t4 = g.add(rt[:, i, idx4[0]:idx4[0] + 1], rt[:, i, idx4[1]:idx4[1] + 1])
                t4 = g.add(t4, rt[:, i, idx4[2]:idx4[2] + 1])
                t4 = g.add(t4, rt[:, i, idx4[3]:idx4[3] + 1])
                t2 = g.add(rt[:, i, idx2[0]:idx2[0] + 1], rt[:, i, idx2[1]:idx2[1] + 1])
                # fc = t4/4/2 + t2/2/2
                o = g.t()
                nc.vector.tensor_scalar_mul(out=o, in0=t4, scalar1=0.125)
                nc.vector.scalar_tensor_tensor(
                    out=fc[:, i:i + 1], in0=t2, scalar=0.25, in1=o,
                    op0=ALU.mult, op1=ALU.add,
                )

            # gaze intersection (Cramer)
            lc, rc = centers["l"], centers["r"]
            lg, rg = gaze["l"], gaze["r"]
            d = [g.sub(rc[:, i:i + 1], lc[:, i:i + 1]) for i in range(3)]
            c0 = [lg[:, i:i + 1] for i in range(3)]
            c1 = [g.mul(rg[:, i:i + 1], consts_neg1 := None) for i in range(0)]  # placeholder
            c1 = []
            for i in range(3):
                o = g.t()
                nc.vector.tensor_scalar_mul(out=o, in0=rg[:, i:i + 1], scalar1=-1.0)
                c1.append(o)
            # c2 = rg x lg
            c2 = list(g.cross3(rg[:, 0:1], rg[:, 1:2], rg[:, 2:3],
                               lg[:, 0:1], lg[:, 1:2], lg[:, 2:3]))
            # w = c1 x c2 ; det = c0.w ; num0 = d.w
            w = g.cross3(*c1, *c2)
            det = g.dot3(*c0, *w)
            num0 = g.dot3(*d, *w)
            # w2 = d x c2 ; num1 = c0.w2  (det with col1 replaced by d)
            w2 = g.cross3(*d, *c2)
            num1 = g.dot3(*c0, *w2)
            rdet = g.t()
            nc.vector.reciprocal(out=rdet, in_=det)
            sol0 = g.mul(num0, rdet)
            sol1 = g.mul(num1, rdet)
            # gp_l = l_c + sol0*lg ; gp_r = r_c + sol1*rg ; gp_mid
            gpl = geop.tile([B, 3], F32)
            gpr = geop.tile([B, 3], F32)
            gpm = geop.tile([B, 3], F32)
            for i in range(3):
                nc.vector.scalar_tensor_tensor(
                    out=gpl[:, i:i + 1], in0=lg[:, i:i + 1], scalar=sol0,
                    in1=lc[:, i:i + 1], op0=ALU.mult, op1=ALU.add,
                )
                nc.vector.scalar_tensor_tensor(
                    out=gpr[:, i:i + 1], in0=rg[:, i:i + 1], scalar=sol1,
                    in1=rc[:, i:i + 1], op0=ALU.mult, op1=ALU.add,
                )
            nc.vector.tensor_tensor(out=gpm, in0=gpl, in1=gpr, op=ALU.add)
            nc.vector.tensor_scalar_mul(out=gpm, in0=gpm, scalar1=0.5)
            dff = geop.tile([B, 3], F32)
            nc.vector.tensor_tensor(out=dff, in0=gpl, in1=gpr, op=ALU.subtract)
            nc.vector.tensor_tensor(out=dff, in0=dff, in1=dff, op=ALU.mult)
            d2 = g.t()
            nc.vector.tensor_reduce(out=d2, in_=dff, axis=AX.X, op=ALU.add)
            dist = g.t()
            nc.scalar.activation(out=dist, in_=d2, func=ACTF.Sqrt)
            # far points l_c + 1000*lg
            farl = geop.tile([B, 3], F32)
            farr = geop.tile([B, 3], F32)
            for i in range(3):
                nc.vector.scalar_tensor_tensor(
                    out=farl[:, i:i + 1], in0=lg[:, i:i + 1], scalar=1000.0,
                    in1=lc[:, i:i + 1], op0=ALU.mult, op1=ALU.add,
                )
                nc.vector.scalar_tensor_tensor(
                    out=farr[:, i:i + 1], in0=rg[:, i:i + 1], scalar=1000.0,
                    in1=rc[:, i:i + 1], op0=ALU.mult, op1=ALU.add,
                )

            # projection of face verts
            cam = geop.tile([B, 12], F32)
            nc.sync.dma_start(out=cam, in_=cam_p[:, :])
            with tc.tile_pool(name="imgp", bufs=1) as imgp:
                img = imgp.tile([B, 3, VM], F32)
                for i in range(3):
                    nc.vector.tensor_scalar(
                        out=img[:, i, :], in0=rt[:, 0, 0:VM],
                        scalar1=cam[:, 4 * i:4 * i + 1], scalar2=cam[:, 4 * i + 3:4 * i + 4],
                        op0=ALU.mult, op1=ALU.add,
                    )
                    for l in (1, 2):
                        nc.vector.scalar_tensor_tensor(
                            out=img[:, i, :], in0=rt[:, l, 0:VM],
                            scalar=cam[:, 4 * i + l:4 * i + l + 1], in1=img[:, i, :],
                            op0=ALU.mult, op1=ALU.add,
                        )
                with tc.tile_pool(name="ztmp", bufs=1) as ztp:
                    az_ = ztp.tile([B, VM], F32)
                    nc.scalar.activation(out=az_, in_=img[:, 2, :], func=ACTF.Abs)
                    nc.vector.tensor_scalar_max(out=az_, in0=az_, scalar1=1e-3)
                    sg = ztp.tile([B, VM], F32)
                    nc.vector.tensor_scalar(
                        out=sg, in0=img[:, 2, :], scalar1=0.0, scalar2=None, op0=ALU.is_ge
                    )
                    nc.vector.tensor_scalar(
                        out=sg, in0=sg, scalar1=2.0, scalar2=1.0,
                        op0=ALU.mult, op1=ALU.subtract,
                    )
                    nc.vector.tensor_tensor(out=sg, in0=sg, in1=az_, op=ALU.mult)
                    rz = ztp.tile([B, VM], F32)
                    nc.vector.reciprocal(out=rz, in_=sg)
                    nc.vector.tensor_tensor(
                        out=img[:, 0, :], in0=img[:, 0, :], in1=rz, op=ALU.mult
                    )
                    nc.vector.tensor_tensor(
                        out=img[:, 1, :], in0=img[:, 1, :], in1=rz, op=ALU.mult
                    )

                # landmark gather + tail assembly
                fl = geop.tile([B, 3, 68], F32)
                engines = [nc.vector, nc.scalar, nc.gpsimd]
                for j, idx in enumerate(fl_idx):
                    for i in range(3):
                        engines[(j * 3 + i) % 3].tensor_copy(
                            out=fl[:, i, j:j + 1], in_=rt[:, i, idx:idx + 1]
                        )
                tail = geop.tile([B, 3, 11], F32)
                for i in range(3):
                    pieces = [
                        lc[:, i:i + 1], rc[:, i:i + 1], fc[:, i:i + 1],
                        gpl[:, i:i + 1], gpr[:, i:i + 1], gpm[:, i:i + 1],
                        farl[:, i:i + 1], farr[:, i:i + 1],
                        lg[:, i:i + 1], rg[:, i:i + 1], dist,
                    ]
                    for j, src in enumerate(pieces):
                        engines[(i * 11 + j) % 3].tensor_copy(
                            out=tail[:, i, j:j + 1], in_=src
                        )

                # output DMAs
                for i in range(3):
                    nc.sync.dma_start(out=out_p[:, i, 0:VM], in_=rt[:, i, 0:VM])
                    nc.sync.dma_start(out=out_p[:, i, VM:2 * VM], in_=img[:, i, :])
                    nc.sync.dma_start(
                        out=out_p[:, i, 2 * VM:2 * VM + 68], in_=fl[:, i, :]
                    )
                    nc.sync.dma_start(
                        out=out_p[:, i, 2 * VM + 68:NOUT], in_=tail[:, i, :]
                    )
    return nc


def _prep(inputs):
    x = np.ascontiguousarray(inputs["x"].reshape(B, DIN), dtype=np.float32)
    enc_W = np.asarray(inputs["enc_W"], dtype=np.float32)
    enc_b = np.asarray(inputs["enc_b"], dtype=np.float32).reshape(1, LAT)
    tmpl = np.ascontiguousarray(
        np.asarray(inputs["v_template"], dtype=np.float32).T
    )  # [3, V]
    basis = np.ascontiguousarray(
        np.asarray(inputs["shape_basis"], dtype=np.float32).transpose(0, 2, 1)
    )  # [400, 3, V]
    cam = np.ascontiguousarray(
        np.asarray(inputs["camera_parameters"], dtype=np.float32).reshape(B, 12)
    )
    lm = np.asarray(inputs["landmarks"])
    mlm = np.asarray(inputs["masked_landmarks"])
    fmask = np.asarray(inputs["face_mask"])
    lmask = np.asarray(inputs["left_eyeball_mask"])
    rmask = np.asarray(inputs["right_eyeball_mask"])
    assert np.array_equal(lmask, np.arange(lmask[0], lmask[0] + 546)), "lmask not contiguous"
    assert np.array_equal(rmask, np.arange(rmask[0], rmask[0] + 546)), "rmask not contiguous"
    fl_idx = [int(fmask[i]) for i in mlm]
    idx4 = [int(lm[j]) for j in (19, 22, 25, 28)]
    idx2 = [int(lm[j]) for j in (14, 18)]
    return (x, enc_W, enc_b, tmpl, basis, cam, fl_idx, idx4, idx2,
            int(lmask[0]), int(rmask[0]))


def _run(inputs, trace=False):
    (x, enc_W, enc_b, tmpl, basis, cam, fl_idx, idx4, idx2, l_lo, r_lo) = _prep(inputs)
    nc = build_graph(fl_idx, idx4, idx2, l_lo, r_lo)
    in_maps = []
    for c in range(NCORES):
        k0 = c * KSH
        in_maps.append({
            "x_sh": np.ascontiguousarray(x[:, k0:k0 + KSH]),
            "w_sh": np.ascontiguousarray(enc_W[k0:k0 + KSH, :]),
            "enc_b": enc_b,
            "tmpl": tmpl,
            "basis": basis,
            "cam": cam,
        })
    res = run_bass_kernel_spmd(
        nc, in_maps, core_ids=list(range(NCORES)), trace=trace
    )
    out = res.results[0]["out"]  # [B, 3, NOUT]
    return np.ascontiguousarray(out.transpose(0, 2, 1)), res


def kernel(**inputs):
    out, _ = _run(inputs, trace=False)
    return out


# revision 25
# speedup vs baseline: 1.0020x; 1.0020x over previous
"""Trainium2 Bass kernel for nn_Autoencoder_65223373357102 (FLAME-style autoencoder).

Strategy:
  Phase 1 (8-way tensor parallel): encoder GEMM [64,150528]@[150528,556] sharded
  along the input-feature axis. Each core transposes its x shard on TensorE,
  multiplies against its 1/8 slice of enc_W, adds enc_b/8 via a K=1 matmul, and
  AllReduces the [64,556] latent (142 KB).
  Phase 2 (replicated): blendshape GEMM [64,400]@[400,3*5023] in plane-separated
  layout + all per-batch geometry with batch on partitions; per-batch scalars are
  broadcast along the free axis via tensor_scalar. Every core computes the full
  output; the host takes core 0's copy.
"""
import sys
import types

sys.path.insert(0, "/opt/trn_rl_repo")

import numpy as np


def _ensure_ntff_hook():
    """Provide antenv.axon_hooks + install the ctypes NTFF profile hook so
    run_bass_kernel_spmd(trace=True) can pull a neuron-profile under axon."""
    name = "antenv.axon_hooks"
    if name not in sys.modules:
        mod = types.ModuleType(name)
        mod._HOOK = None

        def set_axon_ntff_profile_hook(hook):
            mod._HOOK = hook

        def get_axon_ntff_profile_hook():
            return mod._HOOK

        mod.set_axon_ntff_profile_hook = set_axon_ntff_profile_hook
        mod.get_axon_ntff_profile_hook = get_axon_ntff_profile_hook
        sys.modules[name] = mod
        try:
            import antenv

            antenv.axon_hooks = mod
        except ImportError:
            pass
    mod = sys.modules[name]
    if mod.get_axon_ntff_profile_hook() is None:
        try:
            from trn_agent_boot.trn_boot import _ntff_profile_via_ctypes

            hook = _ntff_profile_via_ctypes("/opt/axon/libaxon_pjrt.so")
            if hook is not None:
                mod.set_axon_ntff_profile_hook(hook)
        except Exception:
            pass


_ensure_ntff_hook()

from concourse import bass, mybir, tile
from concourse.bass_utils import run_bass_kernel_spmd

F32 = mybir.dt.float32
ALU = mybir.AluOpType
ACTF = mybir.ActivationFunctionType
AX = mybir.AxisListType

B = 64
V = 5023
VM = 3500
LAT = 556
DIN = 3 * 224 * 224  # 150528
NCORES = 8
KSH = DIN // NCORES  # 18816
KTILES = KSH // 128  # 147
NOUT = 2 * VM + 68 + 11  # 7079
GAZE_DIR = -1.0
HALF_PI = 1.5707963267948966


def _chunks(total, step):
    out = []
    o = 0
    while o < total:
        out.append((o, min(step, total - o)))
        o += step
    return out


class Geo:
    """Helper for tiny per-batch scalar ops on [B,1] tiles."""

    def __init__(self, nc, pool):
        self.nc = nc
        self.pool = pool
        self.n = 0

    def t(self, cols=1):
        self.n += 1
        return self.pool.tile([B, cols], F32, name=f"g{self.n}_{cols}")

    def mul(self, a, b):
        o = self.t()
        self.nc.vector.tensor_tensor(out=o, in0=a, in1=b, op=ALU.mult)
        return o

    def add(self, a, b):
        o = self.t()
        self.nc.vector.tensor_tensor(out=o, in0=a, in1=b, op=ALU.add)
        return o

    def sub(self, a, b):
        o = self.t()
        self.nc.vector.tensor_tensor(out=o, in0=a, in1=b, op=ALU.subtract)
        return o

    def mac(self, a, s, acc):
        """(a * s) + acc, s is a [B,1] AP scalar."""
        o = self.t()
        self.nc.vector.scalar_tensor_tensor(
            out=o, in0=a, scalar=s, in1=acc, op0=ALU.mult, op1=ALU.add
        )
        return o

    def dot3(self, ax, ay, az, bx, by, bz):
        o = self.mul(ax, bx)
        o = self.mac(ay, by, o)
        o = self.mac(az, bz, o)
        return o

    def cross3(self, ax, ay, az, bx, by, bz):
        """a x b -> 3 [B,1] tiles."""
        cx = self.sub(self.mul(ay, bz), self.mul(az, by))
        cy = self.sub(self.mul(az, bx), self.mul(ax, bz))
        cz = self.sub(self.mul(ax, by), self.mul(ay, bx))
        return cx, cy, cz


def axis_angle_R(nc, g, aa3, pfx, halfpi):
    """aa3: [B,3] axis-angle tile -> R [B,9] tile, R[l,i] at col l*3+i.

    R = c*I + s*K + (1-c) a a^T  (Rodrigues, matching reference)
    """
    pool = g.pool
    sq = pool.tile([B, 3], F32, name=pfx + "aaR_sq")
    nc.vector.tensor_tensor(out=sq, in0=aa3, in1=aa3, op=ALU.mult)
    th2 = g.t()
    nc.vector.tensor_reduce(out=th2, in_=sq, axis=AX.X, op=ALU.add)
    theta = g.t()
    nc.scalar.activation(out=theta, in_=th2, func=ACTF.Sqrt)
    thm = g.t()
    nc.vector.tensor_scalar_max(out=thm, in0=theta, scalar1=1e-8)
    rth = g.t()
    nc.vector.reciprocal(out=rth, in_=thm)
    axis3 = pool.tile([B, 3], F32, name=pfx + "aaR_axis")
    nc.vector.tensor_scalar_mul(out=axis3, in0=aa3, scalar1=rth)
    s = g.t()
    nc.scalar.activation(out=s, in_=theta, func=ACTF.Sin)
    c = g.t()
    nc.scalar.activation(out=c, in_=theta, func=ACTF.Sin, bias=halfpi)
    omc = g.t()
    nc.vector.tensor_scalar(
        out=omc, in0=c, scalar1=-1.0, scalar2=1.0, op0=ALU.mult, op1=ALU.add
    )
    ax, ay, az = axis3[:, 0:1], axis3[:, 1:2], axis3[:, 2:3]
    # diag: omc*a_i^2 + c
    asq = pool.tile([B, 3], F32, name=pfx + "aaR_asq")
    nc.vector.tensor_tensor(out=asq, in0=axis3, in1=axis3, op=ALU.mult)
    R = pool.tile([B, 9], F32, name=pfx + "aaR_R")
    dmul = pool.tile([B, 3], F32, name=pfx + "aaR_dmul")
    nc.vector.tensor_scalar_mul(out=dmul, in0=asq, scalar1=omc)
    # s*a
    sa = pool.tile([B, 3], F32, name=pfx + "aaR_sa")
    nc.vector.tensor_scalar_mul(out=sa, in0=axis3, scalar1=s)
    sax, say, saz = sa[:, 0:1], sa[:, 1:2], sa[:, 2:3]
    # off-diag products omc*ai*aj
    mxy = g.mul(g.mul(ax, ay), omc)
    mxz = g.mul(g.mul(ax, az), omc)
    myz = g.mul(g.mul(ay, az), omc)
    # assemble diag: R[l*4] = dmul_l + c
    for l in range(3):
        nc.vector.tensor_tensor(
            out=R[:, 4 * l:4 * l + 1], in0=dmul[:, l:l + 1], in1=c, op=ALU.add
        )
    nc.vector.tensor_tensor(out=R[:, 1:2], in0=mxy, in1=saz, op=ALU.subtract)  # R01
    nc.vector.tensor_tensor(out=R[:, 2:3], in0=mxz, in1=say, op=ALU.add)  # R02
    nc.vector.tensor_tensor(out=R[:, 3:4], in0=mxy, in1=saz, op=ALU.add)  # R10
    nc.vector.tensor_tensor(out=R[:, 5:6], in0=myz, in1=sax, op=ALU.subtract)  # R12
    nc.vector.tensor_tensor(out=R[:, 6:7], in0=mxz, in1=say, op=ALU.subtract)  # R20
    nc.vector.tensor_tensor(out=R[:, 7:8], in0=myz, in1=sax, op=ALU.add)  # R21
    return R


_ENG_ATTR = {
    "SP": "sync", "Pool": "gpsimd", "PE": "tensor",
    "DVE": "vector", "Activation": "scalar",
}


def _legalize_waits(nc):
    """This walrus accepts only one sync-wait slot per instruction; move extra
    waits onto same-engine NoOps inserted right before the instruction."""
    import concourse.mybir as _mybir

    def make_nop(engine):
        eng = getattr(nc, _ENG_ATTR[engine.name])
        bi = eng.nop(nofuse=True)
        mi = bi.ins
        for bb in nc.main_func.blocks:
            if bb.instructions and bb.instructions[-1].name == mi.name:
                bb.instructions.pop()
                break
        mi.engine = engine
        return mi

    for bb in nc.main_func.blocks:
        snapshot = list(bb.instructions)
        newlist = []
        changed = False
        for inst in snapshot:
            si = inst.sync_info
            waits = list(si.on_wait) if (si and si.on_wait) else []
            if (
                len(waits) > 1
                and not inst.name.startswith("barrier")
                and inst.engine is not None
                and getattr(inst.engine, "name", None) in _ENG_ATTR
            ):
                for w in waits[:-1]:
                    nop = make_nop(inst.engine)
                    nop.sync_info = _mybir.SyncInfo(on_wait=[w], on_update=[])
                    newlist.append(nop)
                inst.sync_info = _mybir.SyncInfo(
                    on_wait=[waits[-1]], on_update=list(si.on_update)
                )
                changed = True
            newlist.append(inst)
        if changed:
            bb.instructions[:] = newlist


def build_graph(fl_idx, idx4, idx2, l_lo, r_lo):
    """fl_idx: 68 ints (vert cols for masked landmarks), idx4/idx2: landmark vert
    cols, l_lo/r_lo: start of the contiguous eye ranges."""
    nc = bass.Bass(target_bir_lowering=False)

    x_p = nc.declare_dram_parameter("x_sh", [KSH, B], F32, isOutput=False)
    w_p = nc.declare_dram_parameter("w_sh", [KSH, LAT], F32, isOutput=False)
    b_p = nc.declare_dram_parameter("enc_b", [1, LAT + 128], F32, isOutput=False)
    tpl_p = nc.declare_dram_parameter("tmpl", [3, V], F32, isOutput=False)
    bas_p = nc.declare_dram_parameter("basis", [400, 3, V], F32, isOutput=False)
    cam_p = nc.declare_dram_parameter("cam", [B, 12], F32, isOutput=False)
    out_p = nc.declare_dram_parameter("out", [B, 3, NOUT], F32, isOutput=True)
    dbg_p = nc.declare_dram_parameter("dbg", [B, 6, V], F32, isOutput=True)

    ar_in = nc.dram_tensor("ar_in", [B, LAT], F32)
    ar_out = nc.dram_tensor("ar_out", [B, LAT], F32, addr_space="Shared")

    with tile.TileContext(nc) as tc:
        with (
            tc.tile_pool(name="consts", bufs=1) as consts,
            tc.tile_pool(name="latents", bufs=1) as latp,
            tc.tile_pool(name="geo", bufs=1) as geop,
            tc.tile_pool(name="planes", bufs=1) as planep,
            tc.tile_pool(name="dum", bufs=1, space="PSUM") as dum,
        ):
            b_sb = consts.tile([1, LAT + 128], F32)
            nc.sync.dma_start(out=b_sb, in_=b_p[:, :])
            ones8 = b_sb[:, LAT:LAT + B]       # value 1/NCORES, packed by host
            ones1 = b_sb[:, LAT + B:LAT + 2 * B]  # value 1.0, packed by host
            halfpi = consts.tile([B, 1], F32)
            nc.vector.memset(halfpi, HALF_PI)
            # PE matmuls carry a single sync-wait slot on this walrus; dummy
            # 1-wait matmuls make PE observe one dep before the real matmul.
            d1 = dum.tile([1, 1], F32)
            d64 = dum.tile([B, 1], F32)

            # ---------------- Phase 1: encoder GEMM ----------------
            NSPL = [(0, 512), (512, 44)]
            TPC = 7  # k-tiles per x chunk
            with (
                tc.tile_pool(name="xin", bufs=3) as xin,
                tc.tile_pool(name="wts", bufs=6) as wts,
                tc.tile_pool(name="encp", bufs=1, space="PSUM") as encp,
            ):
                pe = [encp.tile([B, n], F32, name=f"pe{j}", tag=f"pe{j}") for j, (_, n) in enumerate(NSPL)]
                x_view = x_p.ap().rearrange("(c t p) m -> c p t m", t=TPC, p=128)
                for ci in range(KTILES // TPC):
                    x_c = xin.tile([128, TPC, B], F32)
                    nc.sync.dma_start(out=x_c, in_=x_view[ci])
                    nc.tensor.matmul(
                        d1, lhsT=x_c[:, 0, 0:1], rhs=x_c[:, 0, 0:1],
                        start=True, stop=True, skip_group_check=True,
                    )
                    for t in range(TPC):
                        k = ci * TPC + t
                        w_t = wts.tile([128, LAT], F32)
                        nc.sync.dma_start(
                            out=w_t, in_=w_p[k * 128:(k + 1) * 128, :]
                        )
                        for j, (n0, n) in enumerate(NSPL):
                            nc.tensor.matmul(
                                pe[j],
                                lhsT=x_c[:, t, :],
                                rhs=w_t[:, n0:n0 + n],
                                start=(k == 0),
                                stop=False,
                            )
                for j, (n0, n) in enumerate(NSPL):
                    nc.tensor.matmul(
                        pe[j],
                        lhsT=ones8,
                        rhs=b_sb[:, n0:n0 + n],
                        start=False,
                        stop=True,
                    )
                lat1 = latp.tile([B, LAT], F32)
                for j, (n0, n) in enumerate(NSPL):
                    nc.vector.tensor_copy(out=lat1[:, n0:n0 + n], in_=pe[j])
                nc.sync.dma_start(out=ar_in[:, :], in_=lat1)

            nc.gpsimd.collective_compute(
                "AllReduce",
                ALU.add,
                replica_groups=[list(range(NCORES))],
                ins=[ar_in.ap().opt()],
                outs=[ar_out.ap().opt()],
            )
            lat = latp.tile([B, LAT], F32)
            nc.sync.dma_start(out=lat, in_=ar_out[:, :])

            # ---------------- Phase 1.5: transpose shape params ----------------
            # DVE 32x32 block transposes: spT[ki][r, b] = lat[b, c0+r].
            # kw = rows used by the matmul; alloc rows padded to 32 (the pad
            # reads harmless latent cols >=400).
            KSPL = [(0, 128, 128), (128, 128, 128), (256, 128, 128), (384, 16, 32)]
            spT = []
            for (c0, kw, cwa) in KSPL:
                st = latp.tile([cwa, B], F32, name=f"spT{c0}", tag=f"spT{c0}")
                for pb in range(cwa // 32):
                    for fb in range(B // 32):
                        nc.vector.transpose(
                            out=st[32 * pb:32 * pb + 32, 32 * fb:32 * fb + 32],
                            in_=lat[32 * fb:32 * fb + 32,
                                    c0 + 32 * pb:c0 + 32 * pb + 32],
                        )
                spT.append(st)
            nc.tensor.matmul(
                d64, lhsT=spT[3], rhs=spT[3][:, 0:1],
                start=True, stop=True, skip_group_check=True,
            )

            # ---------------- Phase 2: blendshape + template ----------------
            vertp_ctx = tc.tile_pool(name="vertp", bufs=1)
            vertp = vertp_ctx.__enter__()
            vert = vertp.tile([B, 3, V], F32)  # raw verts, plane-major
            vps = geop.tile([B, 33], F32)  # per-chunk partial sums
            VCH = _chunks(V, 512)  # 10 chunks
            with (
                tc.tile_pool(name="bas", bufs=4) as basp,
                tc.tile_pool(name="tpl", bufs=2) as tplp,
                tc.tile_pool(name="bpsum", bufs=2, space="PSUM") as bpsum,
            ):
                done = []  # (p, n0) of completed chunks, for WAR-absorbing dummies
                for p in range(3):
                    for j, (n0, n) in enumerate(VCH):
                        if len(done) >= 2:
                            pp, pn0 = done[-2]
                            nc.tensor.matmul(
                                d1, lhsT=vert[:, pp, pn0:pn0 + 1],
                                rhs=vert[:, pp, pn0:pn0 + 1],
                                start=True, stop=True, skip_group_check=True,
                            )
                        pv = bpsum.tile([B, 512], F32)
                        for ki, (k0, kw, _cwa) in enumerate(KSPL):
                            bt = basp.tile([128, 512], F32)
                            nc.sync.dma_start(
                                out=bt[:kw, :n], in_=bas_p[k0:k0 + kw, p, n0:n0 + n]
                            )
                            nc.tensor.matmul(
                                pv[:, :n],
                                lhsT=spT[ki][:kw, :],
                                rhs=bt[:kw, :n],
                                start=(ki == 0),
                                stop=False,
                            )
                        tl = tplp.tile([1, 512], F32)
                        nc.sync.dma_start(out=tl[:, :n], in_=tpl_p[p:p + 1, n0:n0 + n])
                        nc.tensor.matmul(
                            pv[:, :n], lhsT=ones1, rhs=tl[:, :n], start=False, stop=True
                        )
                        nc.scalar.copy(out=vert[:, p, n0:n0 + n], in_=pv[:, :n])
                        nc.vector.tensor_reduce(
                            out=vps[:, p * 11 + j:p * 11 + j + 1],
                            in_=vert[:, p, n0:n0 + n],
                            axis=AX.X,
                            op=ALU.add,
                        )
                        done.append((p, n0))

            g = Geo(nc, geop)
            # vmean [B,3]
            vm = geop.tile([B, 3], F32)
            NCH = len(VCH)
            for p in range(3):
                nc.vector.tensor_reduce(
                    out=vm[:, p:p + 1], in_=vps[:, p * 11:p * 11 + NCH],
                    axis=AX.X, op=ALU.add,
                )
            vms = geop.tile([B, 3], F32)
            nc.vector.tensor_scalar_mul(out=vms, in0=vm, scalar1=1.0 / V)

            # face rotation matrix, scaled
            aa_face = lat[:, 545:548]
            Rf = axis_angle_R(nc, g, aa_face, "f_", halfpi)
            fs = g.t()  # face_scale = latent[551]+1
            nc.vector.tensor_scalar_add(out=fs, in0=lat[:, 551:552], scalar1=1.0)
            Rs = geop.tile([B, 9], F32)
            nc.vector.tensor_scalar_mul(out=Rs, in0=Rf, scalar1=fs)
            # offsets: off_i = face_t_i - sum_l vms_l*Rs[l,i]
            off = geop.tile([B, 3], F32)
            for i in range(3):
                t = g.mul(vms[:, 0:1], Rs[:, i:i + 1])
                t = g.mac(vms[:, 1:2], Rs[:, 3 + i:4 + i], t)
                t = g.mac(vms[:, 2:3], Rs[:, 6 + i:7 + i], t)
                nc.vector.tensor_tensor(
                    out=off[:, i:i + 1], in0=lat[:, 548 + i:549 + i], in1=t,
                    op=ALU.subtract,
                )

            # rotate+scale+translate all verts: rt[:,i,:] = sum_l vert[:,l,:]*Rs[l,i] + off_i
            rt = planep.tile([B, 3, V], F32)
            for i in range(3):
                nc.vector.tensor_scalar(
                    out=rt[:, i, :], in0=vert[:, 0, :],
                    scalar1=Rs[:, i:i + 1], scalar2=off[:, i:i + 1],
                    op0=ALU.mult, op1=ALU.add,
                )
                for l in (1, 2):
                    nc.vector.scalar_tensor_tensor(
                        out=rt[:, i, :], in0=vert[:, l, :],
                        scalar=Rs[:, 3 * l + i:3 * l + i + 1], in1=rt[:, i, :],
                        op0=ALU.mult, op1=ALU.add,
                    )
            nc.sync.dma_start(out=dbg_p[:, 0:3, :], in_=vert)
            nc.sync.dma_start(out=dbg_p[:, 3:6, :], in_=rt)
            vertp_ctx.__exit__(None, None, None)

            # eye processing
            EW = 546
            gaze = {}
            centers = {}
            for side, lo, pivot, rcol in (
                ("l", l_lo, 4051, 552), ("r", r_lo, 4597, 554),
            ):
                # center = mean of rotated eye verts
                cc = geop.tile([B, 3], F32, name=f"cc_{side}")
                for i in range(3):
                    nc.vector.tensor_reduce(
                        out=cc[:, i:i + 1], in_=rt[:, i, lo:lo + EW],
                        axis=AX.X, op=ALU.add,
                    )
                c3 = geop.tile([B, 3], F32, name=f"c3_{side}")
                nc.vector.tensor_scalar_mul(out=c3, in0=cc, scalar1=1.0 / EW)
                centers[side] = c3
                # init gaze dir = normalize(vert[pivot] - c)
                a3 = geop.tile([B, 3], F32, name=f"a3_{side}")
                for i in range(3):
                    nc.vector.tensor_tensor(
                        out=a3[:, i:i + 1], in0=rt[:, i, pivot:pivot + 1],
                        in1=c3[:, i:i + 1], op=ALU.subtract,
                    )
                sq = geop.tile([B, 3], F32, name=f"nsq_{side}")
                nc.vector.tensor_tensor(out=sq, in0=a3, in1=a3, op=ALU.mult)
                n2 = g.t()
                nc.vector.tensor_reduce(out=n2, in_=sq, axis=AX.X, op=ALU.add)
                nn = g.t()
                nc.scalar.activation(out=nn, in_=n2, func=ACTF.Sqrt)
                rn = g.t()
                nc.vector.reciprocal(out=rn, in_=nn)
                nc.vector.tensor_scalar_mul(out=a3, in0=a3, scalar1=rn)
                ax, ay, az = a3[:, 0:1], a3[:, 1:2], a3[:, 2:3]
                # find_gaze_R: b=(0,0,GAZE_DIR); v = a x b = (ay*g, -ax*g, 0)
                vx = g.t()
                nc.vector.tensor_scalar_mul(out=vx, in0=ay, scalar1=GAZE_DIR)
                vy = g.t()
                nc.vector.tensor_scalar_mul(out=vy, in0=ax, scalar1=-GAZE_DIR)
                # c = a.b = az*g
                cdot = g.t()
                nc.vector.tensor_scalar_mul(out=cdot, in0=az, scalar1=GAZE_DIR)
                # f = 1/(1+c+1e-8)
                fden = g.t()
                nc.vector.tensor_scalar_add(out=fden, in0=cdot, scalar1=1.0 + 1e-8)
                f = g.t()
                nc.vector.reciprocal(out=f, in_=fden)
                # vv = vx^2+vy^2
                vv = g.mac(vy, vy, g.mul(vx, vx))
                fvv = g.mul(f, vv)
                dd = g.t()  # 1 - f*vv
                nc.vector.tensor_scalar(
                    out=dd, in0=fvv, scalar1=-1.0, scalar2=1.0, op0=ALU.mult, op1=ALU.add
                )
                fxy = g.mul(g.mul(vx, vy), f)
                Rl = geop.tile([B, 9], F32, name=f"Rl_{side}")
                # Rl00 = dd + f*vx^2 ; Rl11 = dd + f*vy^2; Rl22 = dd
                nc.vector.tensor_tensor(
                    out=Rl[:, 0:1], in0=dd, in1=g.mul(f, g.mul(vx, vx)), op=ALU.add
                )
                nc.vector.tensor_tensor(
                    out=Rl[:, 4:5], in0=dd, in1=g.mul(f, g.mul(vy, vy)), op=ALU.add
                )
                nc.vector.tensor_copy(out=Rl[:, 8:9], in_=dd)
                nc.vector.tensor_copy(out=Rl[:, 1:2], in_=fxy)  # R01 = f vx vy
                nc.vector.tensor_copy(out=Rl[:, 3:4], in_=fxy)  # R10
                nc.vector.tensor_copy(out=Rl[:, 2:3], in_=vy)  # R02 = vy
                nc.vector.tensor_scalar_mul(out=Rl[:, 5:6], in0=vx, scalar1=-1.0)  # R12
                nc.vector.tensor_scalar_mul(out=Rl[:, 6:7], in0=vy, scalar1=-1.0)  # R20
                nc.vector.tensor_copy(out=Rl[:, 7:8], in_=vx)  # R21
                # eyeball rotation from latent rot2 (2 comps, az=0)
                aa2 = geop.tile([B, 3], F32, name=f"aa2_{side}")
                nc.vector.memset(aa2, 0.0)
                nc.vector.tensor_copy(out=aa2[:, 0:2], in_=lat[:, rcol:rcol + 2])
                R2 = axis_angle_R(nc, g, aa2, side + "_", halfpi)
                # gaze = GAZE_DIR * R2[2,:]
                gz = geop.tile([B, 3], F32, name=f"gz_{side}")
                nc.vector.tensor_scalar_mul(out=gz, in0=R2[:, 6:9], scalar1=GAZE_DIR)
                gaze[side] = gz
                # M = Rl @ R2
                M = geop.tile([B, 9], F32, name=f"M_{side}")
                for l in range(3):
                    for i in range(3):
                        t = g.mul(Rl[:, 3 * l:3 * l + 1], R2[:, i:i + 1])
                        t = g.mac(R2[:, 3 + i:4 + i], Rl[:, 3 * l + 1:3 * l + 2], t)
                        t = g.mac(R2[:, 6 + i:7 + i], Rl[:, 3 * l + 2:3 * l + 3], t)
                        nc.vector.tensor_copy(out=M[:, 3 * l + i:3 * l + i + 1], in_=t)
                # offe_i = c_i - sum_l c_l M[l,i]
                offe = geop.tile([B, 3], F32, name=f"offe_{side}")
                for i in range(3):
                    t = g.mul(c3[:, 0:1], M[:, i:i + 1])
                    t = g.mac(c3[:, 1:2], M[:, 3 + i:4 + i], t)
                    t = g.mac(c3[:, 2:3], M[:, 6 + i:7 + i], t)
                    nc.vector.tensor_tensor(
                        out=offe[:, i:i + 1], in0=c3[:, i:i + 1], in1=t, op=ALU.subtract
                    )
                # apply to eye slice
                etmp = planep.tile([B, 3, EW], F32, name=f"etmp_{side}")
                for i in range(3):
                    nc.vector.tensor_scalar(
                        out=etmp[:, i, :], in0=rt[:, 0, lo:lo + EW],
                        scalar1=M[:, i:i + 1], scalar2=offe[:, i:i + 1],
                        op0=ALU.mult, op1=ALU.add,
                    )
                    for l in (1, 2):
                        nc.vector.scalar_tensor_tensor(
                            out=etmp[:, i, :], in0=rt[:, l, lo:lo + EW],
                            scalar=M[:, 3 * l + i:3 * l + i + 1], in1=etmp[:, i, :],
                            op0=ALU.mult, op1=ALU.add,
                        )
                for i in range(3):
                    nc.vector.tensor_copy(out=rt[:, i, lo:lo + EW], in_=etmp[:, i, :])

            # face centre from landmarks
            fc = geop.tile([B, 3], F32)
            for i in range(3):
                t4 = g.add(rt[:, i, idx4[0]:idx4[0] + 1], rt[:, i, idx4[1]:idx4[1] + 1])
                t4 = g.add(t4, rt[:, i, idx4[2]:idx4[2] + 1])
                t4 = g.add(t4, rt[:, i, idx4[3]:idx4[3] + 1])
                t2 = g.add(rt[:, i, idx2[0]:idx2[0] + 1], rt[:, i, idx2[1]:idx2[1] + 1])
                # fc = t4/4/2 + t2/2/2
                o = g.t()
                nc.vector.tensor_scalar_mul(out=o, in0=t4, scalar1=0.125)
                nc.vector.scalar_tensor_tensor(
                    out=fc[:, i:i + 1], in0=t2, scalar=0.25, in1=o,
                    op0=ALU.mult, op1=ALU.add,
                )

            # gaze intersection (Cramer)
            lc, rc = centers["l"], centers["r"]
            lg, rg = gaze["l"], gaze["r"]
            d = [g.sub(rc[:, i:i + 1], lc[:, i:i + 1]) for i in range(3)]
            c0 = [lg[:, i:i + 1] for i in range(3)]
            c1 = []
            for i in range(3):
                o = g.t()
                nc.vector.tensor_scalar_mul(out=o, in0=rg[:, i:i + 1], scalar1=-1.0)
                c1.append(o)
            # c2 = rg x lg
            c2 = list(g.cross3(rg[:, 0:1], rg[:, 1:2], rg[:, 2:3],
                               lg[:, 0:1], lg[:, 1:2], lg[:, 2:3]))
            # w = c1 x c2 ; det = c0.w ; num0 = d.w
            w = g.cross3(*c1, *c2)
            det = g.dot3(*c0, *w)
            num0 = g.dot3(*d, *w)
            # w2 = d x c2 ; num1 = c0.w2  (det with col1 replaced by d)
            w2 = g.cross3(*d, *c2)
            num1 = g.dot3(*c0, *w2)
            rdet = g.t()
            nc.vector.reciprocal(out=rdet, in_=det)
            sol0 = g.mul(num0, rdet)
            sol1 = g.mul(num1, rdet)
            # gp_l = l_c + sol0*lg ; gp_r = r_c + sol1*rg ; gp_mid
            gpl = geop.tile([B, 3], F32)
            gpr = geop.tile([B, 3], F32)
            gpm = geop.tile([B, 3], F32)
            for i in range(3):
                nc.vector.scalar_tensor_tensor(
                    out=gpl[:, i:i + 1], in0=lg[:, i:i + 1], scalar=sol0,
                    in1=lc[:, i:i + 1], op0=ALU.mult, op1=ALU.add,
                )
                nc.vector.scalar_tensor_tensor(
                    out=gpr[:, i:i + 1], in0=rg[:, i:i + 1], scalar=sol1,
                    in1=rc[:, i:i + 1], op0=ALU.mult, op1=ALU.add,
                )
            nc.vector.tensor_tensor(out=gpm, in0=gpl, in1=gpr, op=ALU.add)
            nc.vector.tensor_scalar_mul(out=gpm, in0=gpm, scalar1=0.5)
            dff = geop.tile([B, 3], F32)
            nc.vector.tensor_tensor(out=dff, in0=gpl, in1=gpr, op=ALU.subtract)
            nc.vector.tensor_tensor(out=dff, in0=dff, in1=dff, op=ALU.mult)
            d2 = g.t()
            nc.vector.tensor_reduce(out=d2, in_=dff, axis=AX.X, op=ALU.add)
            dist = g.t()
            nc.scalar.activation(out=dist, in_=d2, func=ACTF.Sqrt)
            # far points l_c + 1000*lg
            farl = geop.tile([B, 3], F32)
            farr = geop.tile([B, 3], F32)
            for i in range(3):
                nc.vector.scalar_tensor_tensor(
                    out=farl[:, i:i + 1], in0=lg[:, i:i + 1], scalar=1000.0,
                    in1=lc[:, i:i + 1], op0=ALU.mult, op1=ALU.add,
                )
                nc.vector.scalar_tensor_tensor(
                    out=farr[:, i:i + 1], in0=rg[:, i:i + 1], scalar=1000.0,
                    in1=rc[:, i:i + 1], op0=ALU.mult, op1=ALU.add,
                )

            # projection of face verts
            cam = geop.tile([B, 12], F32)
            nc.sync.dma_start(out=cam, in_=cam_p[:, :])
            with tc.tile_pool(name="imgp", bufs=1) as imgp:
                img = imgp.tile([B, 3, VM], F32)
                for i in range(3):
                    nc.vector.tensor_scalar(
                        out=img[:, i, :], in0=rt[:, 0, 0:VM],
                        scalar1=cam[:, 4 * i:4 * i + 1], scalar2=cam[:, 4 * i + 3:4 * i + 4],
                        op0=ALU.mult, op1=ALU.add,
                    )
                    for l in (1, 2):
                        nc.vector.scalar_tensor_tensor(
                            out=img[:, i, :], in0=rt[:, l, 0:VM],
                            scalar=cam[:, 4 * i + l:4 * i + l + 1], in1=img[:, i, :],
                            op0=ALU.mult, op1=ALU.add,
                        )
                with tc.tile_pool(name="ztmp", bufs=1) as ztp:
                    az_ = ztp.tile([B, VM], F32)
                    nc.scalar.activation(out=az_, in_=img[:, 2, :], func=ACTF.Abs)
                    nc.vector.tensor_scalar_max(out=az_, in0=az_, scalar1=1e-3)
                    sg = ztp.tile([B, VM], F32)
                    nc.vector.tensor_scalar(
                        out=sg, in0=img[:, 2, :], scalar1=0.0, scalar2=None, op0=ALU.is_ge
                    )
                    nc.vector.tensor_scalar(
                        out=sg, in0=sg, scalar1=2.0, scalar2=1.0,
                        op0=ALU.mult, op1=ALU.subtract,
                    )
                    nc.vector.tensor_tensor(out=sg, in0=sg, in1=az_, op=ALU.mult)
                    nc.vector.reciprocal(out=az_, in_=sg)
                    nc.vector.tensor_tensor(
                        out=img[:, 0, :], in0=img[:, 0, :], in1=az_, op=ALU.mult
                    )
                    nc.vector.tensor_tensor(
                        out=img[:, 1, :], in0=img[:, 1, :], in1=az_, op=ALU.mult
                    )

                # landmark gather + tail assembly
                fl = geop.tile([B, 3, 68], F32)
                def _cp(k, out, in_):
                    e = k % 3
                    if e == 0:
                        nc.vector.tensor_copy(out=out, in_=in_)
                    elif e == 1:
                        nc.scalar.copy(out=out, in_=in_)
                    else:
                        nc.gpsimd.tensor_copy(out=out, in_=in_)

                for j, idx in enumerate(fl_idx):
                    for i in range(3):
                        _cp(j * 3 + i, fl[:, i, j:j + 1], rt[:, i, idx:idx + 1])
                tail = geop.tile([B, 3, 11], F32)
                for i in range(3):
                    pieces = [
                        lc[:, i:i + 1], rc[:, i:i + 1], fc[:, i:i + 1],
                        gpl[:, i:i + 1], gpr[:, i:i + 1], gpm[:, i:i + 1],
                        farl[:, i:i + 1], farr[:, i:i + 1],
                        lg[:, i:i + 1], rg[:, i:i + 1], dist,
                    ]
                    for j, src in enumerate(pieces):
                        _cp(i * 11 + j, tail[:, i, j:j + 1], src)

                # output DMAs
                for i in range(3):
                    nc.sync.dma_start(out=out_p[:, i, 0:VM], in_=rt[:, i, 0:VM])
                    nc.sync.dma_start(out=out_p[:, i, VM:2 * VM], in_=img[:, i, :])
                    nc.sync.dma_start(
                        out=out_p[:, i, 2 * VM:2 * VM + 68], in_=fl[:, i, :]
                    )
                    nc.sync.dma_start(
                        out=out_p[:, i, 2 * VM + 68:NOUT], in_=tail[:, i, :]
                    )
    _legalize_waits(nc)
    return nc


def _prep(inputs):
    x = np.ascontiguousarray(inputs["x"].reshape(B, DIN), dtype=np.float32)
    enc_W = np.asarray(inputs["enc_W"], dtype=np.float32)
    enc_b = np.concatenate([
        np.asarray(inputs["enc_b"], dtype=np.float32).reshape(1, LAT),
        np.full((1, B), 1.0 / NCORES, np.float32),
        np.ones((1, B), np.float32),
    ], axis=1)
    tmpl = np.ascontiguousarray(
        np.asarray(inputs["v_template"], dtype=np.float32).T
    )  # [3, V]
    basis = np.ascontiguousarray(
        np.asarray(inputs["shape_basis"], dtype=np.float32).transpose(0, 2, 1)
    )  # [400, 3, V]
    cam = np.ascontiguousarray(
        np.asarray(inputs["camera_parameters"], dtype=np.float32).reshape(B, 12)
    )
    lm = np.asarray(inputs["landmarks"])
    mlm = np.asarray(inputs["masked_landmarks"])
    fmask = np.asarray(inputs["face_mask"])
    lmask = np.asarray(inputs["left_eyeball_mask"])
    rmask = np.asarray(inputs["right_eyeball_mask"])
    assert np.array_equal(lmask, np.arange(lmask[0], lmask[0] + 546)), "lmask not contiguous"
    assert np.array_equal(rmask, np.arange(rmask[0], rmask[0] + 546)), "rmask not contiguous"
    fl_idx = [int(fmask[i]) for i in mlm]
    idx4 = [int(lm[j]) for j in (19, 22, 25, 28)]
    idx2 = [int(lm[j]) for j in (14, 18)]
    return (x, enc_W, enc_b, tmpl, basis, cam, fl_idx, idx4, idx2,
            int(lmask[0]), int(rmask[0]))


def _run(inputs, trace=False):
    (x, enc_W, enc_b, tmpl, basis, cam, fl_idx, idx4, idx2, l_lo, r_lo) = _prep(inputs)
    nc = build_graph(fl_idx, idx4, idx2, l_lo, r_lo)
    in_maps = []
    for c in range(NCORES):
        k0 = c * KSH
        in_maps.append({
            "x_sh": np.ascontiguousarray(x[:, k0:k0 + KSH].T),
            "w_sh": np.ascontiguousarray(enc_W[k0:k0 + KSH, :]),
            "enc_b": enc_b,
            "tmpl": tmpl,
            "basis": basis,
            "cam": cam,
        })
    res = run_bass_kernel_spmd(
        nc, in_maps, core_ids=list(range(NCORES)), trace=trace
    )
    out = res.results[0]["out"]  # [B, 3, NOUT]
    return np.ascontiguousarray(out.transpose(0, 2, 1)), res


def kernel(**inputs):
    out, _ = _run(inputs, trace=False)
    return out
